# revision 19
# baseline (speedup 1.0000x reference)
"""Trainium2 Bass kernel for nn_AttentionBlock (GroupNorm + MHA + proj + residual).

Input  x: [16, 512, 32, 32] fp32.  8 NeuronCores, data-parallel over batch
(2 images per core).  Everything is hardcoded for these shapes.

fp8-e4m3 DoubleRow edition.  All matmuls except the score matmul run in
fp8 e4m3 with perf_mode=DoubleRow (K=256 per instruction, 0.5 cyc/col):
  - QKV and proj weights are host-prescaled by 16 (fp8-friendly range)
    and unscaled in the psum evacuation.
  - q,k are evacuated to bf16; the S^T = K^T Q matmul runs in bf16
    (full-rate, contraction d=128 can't double-row without a partition
    remap).
  - exp runs on the Scalar engine with bias -ln(4) so P/4 stays inside
    e4m3 range (max ~240); the /4 cancels in the softmax ratio because
    the SAME fp8 pt tensor feeds both the PV matmul and the rowsum.
  - rowsum via a skinny ones-lhsT DoubleRow matmul (out [1, n]); the
    reciprocal is broadcast to 128 partitions on GpSimd.
  - v is computed pre-transposed [m, c_v] as fp8 with mt-pair planes so
    PV contracts 256 tokens per instruction.
Engine split: exp + q-evac on Scalar; k-evac, v-evac, reciprocal,
ot-normalize, proj-unscale on DVE; GroupNorm apply, rowsum broadcast and
the residual add on GpSimd (no PSUM port there, SBUF-only work).
DMA: x + out on the Act HWDGE queue, weights + stores on the SP queue.

Numerics: scores are ~N(0,1) over 1024 keys => near-flat softmax, so fp8
error in q/k/v/xn washes out in the weighted average; tolerance is 2e-2
and this lands ~1e-3.
"""

import math
import numpy as np

import concourse.bass as bass
import concourse.bacc as bacc
import concourse.tile as tile
from concourse import mybir
from concourse.bass_utils import run_bass_kernel_spmd

N_CORES = 8
B, C, HH, WW = 16, 512, 32, 32
N = HH * WW            # 1024 tokens per image
NH, DH = 4, 128        # heads, head dim
G, GS = 8, 64          # groups, channels per group
B_LOC = B // N_CORES   # images per core
EPS = 1e-5
CT = C // 128          # 4 channel tiles
NT = N // 128          # 8 token tiles
SCALE = float(DH) ** -0.5
WS = 16.0              # host-side weight prescale
EB = -math.log(4.0)    # exp bias: pt = P/4, keeps e4m3 in range
OS = 1.0 / 16.0        # ones value: rowsum psum = sum(pt)/16

f32 = mybir.dt.float32
bf16 = mybir.dt.bfloat16
fp8 = mybir.dt.float8e4
AF = mybir.ActivationFunctionType
OP = mybir.AluOpType
DR = mybir.MatmulPerfMode.DoubleRow


def build_program():
    nc = bacc.Bacc("TRN2", target_bir_lowering=False, debug=False)

    x_d = nc.dram_tensor("x", [B_LOC, C, N], f32, kind="ExternalInput").ap()
    wqk_d = nc.dram_tensor("wqk", [128, 2, 2, 2 * C], fp8, kind="ExternalInput").ap()
    wv_d = nc.dram_tensor("wv", [128, 2, 2, C], fp8, kind="ExternalInput").ap()
    wp_d = nc.dram_tensor("wp", [128, 2, 2, C], fp8, kind="ExternalInput").ap()
    qkb_d = nc.dram_tensor("qkb", [128, 2 * C // 128], f32, kind="ExternalInput").ap()
    vb_d = nc.dram_tensor("vb", [C], f32, kind="ExternalInput").ap()
    pb_d = nc.dram_tensor("pb", [128, CT], f32, kind="ExternalInput").ap()
    gam_d = nc.dram_tensor("gamma", [128, CT], f32, kind="ExternalInput").ap()
    bet_d = nc.dram_tensor("beta", [128, CT], f32, kind="ExternalInput").ap()
    out_d = nc.dram_tensor("out", [B_LOC, C, N], f32, kind="ExternalOutput").ap()

    with tile.TileContext(nc) as tc:
        with (
            tc.tile_pool(name="wpool", bufs=1) as wpool,
            tc.tile_pool(name="xpool", bufs=2) as xpool,
            tc.tile_pool(name="xnpool", bufs=2) as xnpool,
            tc.tile_pool(name="qkpool", bufs=2) as qkpool,
            tc.tile_pool(name="vtpool", bufs=2) as vtpool,
            tc.tile_pool(name="ptpool", bufs=2) as ptpool,
            tc.tile_pool(name="otpool", bufs=2) as otpool,
            tc.tile_pool(name="rpool", bufs=2) as rpool,
            tc.tile_pool(name="tpool", bufs=2) as tpool,
            tc.tile_pool(name="outpool", bufs=2) as outpool,
            tc.tile_pool(name="spool", bufs=2) as spool,
            tc.tile_pool(name="chpool", bufs=4) as chpool,
            tc.tile_pool(name="mmps", bufs=2, space="PSUM") as mmps,
            tc.tile_pool(name="pvps", bufs=1, space="PSUM") as pvps,
            tc.tile_pool(name="rsps", bufs=1, space="PSUM") as rsps,
        ):
            # ---- x0 alone on the Act HWDGE queue (GN0 critical path);
            # wqk first then x1 + the rest on the SP queue ----
            xts = []
            for img in range(B_LOC):
                xt = xpool.tile([128, CT, N], f32, tag="x", name=f"xt{img}")
                xts.append(xt)
            wqk_sb = wpool.tile([128, 2, 2, 2 * C], fp8, tag="wqk")

            xr0 = x_d[0].rearrange("(t p) n -> p t n", p=128)
            for ct in range(CT):
                nc.scalar.dma_start(xts[0][:, ct, :], xr0[:, ct, :])
            nc.sync.dma_start(wqk_sb[:], wqk_d[:])
            xr1 = x_d[1].rearrange("(t p) n -> p t n", p=128)
            for ct in range(CT):
                nc.sync.dma_start(xts[1][:, ct, :], xr1[:, ct, :])
            wv_sb = wpool.tile([128, 2, 2, C], fp8, tag="wv")
            nc.sync.dma_start(wv_sb[:], wv_d[:])
            wp_sb = wpool.tile([128, 2, 2, C], fp8, tag="wp")
            nc.sync.dma_start(wp_sb[:], wp_d[:])

            qkb_sb = wpool.tile([128, 2 * C // 128], f32, tag="qkb")
            nc.sync.dma_start(qkb_sb[:], qkb_d[:])
            pb_sb = wpool.tile([128, CT], f32, tag="pb")
            nc.sync.dma_start(pb_sb[:], pb_d[:])
            gam_sb = wpool.tile([128, CT], f32, tag="gam")
            nc.sync.dma_start(gam_sb[:], gam_d[:])
            bet_sb = wpool.tile([128, CT], f32, tag="bet")
            nc.sync.dma_start(bet_sb[:], bet_d[:])
            # vb broadcast to all partitions, twice along free (nt-pair stt)
            vb_bc2 = wpool.tile([128, 2, C], f32, tag="vbbc")
            nc.sync.dma_start(
                vb_bc2[:],
                bass.AP(tensor=vb_d.tensor, offset=vb_d.offset,
                        ap=[[0, 128], [0, 2], [1, C]]))

            sel = wpool.tile([128, 2], f32, tag="sel")
            nc.vector.memset(sel[0:64, 0:1], 1.0 / GS)
            nc.vector.memset(sel[64:128, 0:1], 0.0)
            nc.vector.memset(sel[0:64, 1:2], 0.0)
            nc.vector.memset(sel[64:128, 1:2], 1.0 / GS)
            # [128, 2, 16]: k-subtile plane step must be 16B-aligned for
            # DoubleRow ldweights; only column 0 is used.
            ones8 = wpool.tile([128, 2, 16], fp8, tag="ones8")
            nc.vector.memset(ones8[:], OS)
            eps_t = wpool.tile([2, 1], f32, tag="eps")
            nc.vector.memset(eps_t[:], EPS)
            eb_t = wpool.tile([128, 1], f32, tag="eb")
            nc.vector.memset(eb_t[:], EB)

            def stage_gn(img):
                """GroupNorm stats on DVE, apply on GpSimd -> xn fp8."""
                xt = xts[img]
                stats2 = spool.tile([128, 2 * CT], f32, tag="stats2",
                                    name=f"stats2_{img}")
                for ct in range(CT):
                    st = spool.tile([128, 2, 6], f32, tag="bnst", name="st")
                    nc.vector.bn_stats(st[:, 0, :], xt[:, ct, 0:512])
                    nc.vector.bn_stats(st[:, 1, :], xt[:, ct, 512:1024])
                    mv = spool.tile([128, 2], f32, tag="mv", name="mv")
                    nc.vector.bn_aggr(mv[:], st[:])
                    nc.vector.tensor_copy(stats2[:, 2 * ct:2 * ct + 1], mv[:, 0:1])
                    nc.vector.tensor_mul(
                        stats2[:, 2 * ct + 1:2 * ct + 2], mv[:, 0:1], mv[:, 0:1])
                    nc.vector.tensor_add(
                        stats2[:, 2 * ct + 1:2 * ct + 2],
                        stats2[:, 2 * ct + 1:2 * ct + 2], mv[:, 1:2])
                psg_t = mmps.tile([128, 2, 512], f32, tag="mm", name="psg")
                psg = psg_t[0:2, 0, 0:2 * CT]
                nc.tensor.matmul(psg, sel[:], stats2[:], start=True, stop=True)
                gs = spool.tile([2, 2 * CT], f32, tag="gs", name="gs")
                nc.scalar.activation(gs[:], psg, AF.Copy)
                gs3 = gs[:].rearrange("p (t s) -> p t s", s=2)
                tmp = spool.tile([2, CT], f32, tag="gtmp", name="tmp")
                nc.vector.tensor_mul(tmp[:], gs3[:, :, 0], gs3[:, :, 0])
                var_g = spool.tile([2, CT], f32, tag="gvar", name="var_g")
                nc.vector.tensor_sub(var_g[:], gs3[:, :, 1], tmp[:])
                sd = spool.tile([2, CT], f32, tag="gsd", name="sd")
                nc.scalar.activation(sd[:], var_g[:], AF.Sqrt, bias=eps_t[:])
                rstd_g = spool.tile([2, CT], f32, tag="grstd", name="rstd_g")
                rscr = spool.tile([2, CT], f32, tag="grscr", name="rscr")
                nc.vector.reciprocal_approx_accurate(rstd_g[:], sd[:], rscr[:])

                xnt = xnpool.tile([128, 2, 2, N], fp8, tag="xn", name=f"xn{img}")
                for ct in range(CT):
                    mu_ch = chpool.tile([128, 1], f32, tag="much", name="mu_ch")
                    sg = gs[:, 2 * ct:2 * ct + 1]
                    nc.sync.dma_start(
                        mu_ch[:],
                        bass.AP(tensor=sg.tensor, offset=sg.offset,
                                ap=[[sg.ap[0][0], 2], [0, GS]]))
                    rs_ch = chpool.tile([128, 1], f32, tag="rsch", name="rs_ch")
                    sg = rstd_g[:, ct:ct + 1]
                    nc.sync.dma_start(
                        rs_ch[:],
                        bass.AP(tensor=sg.tensor, offset=sg.offset,
                                ap=[[sg.ap[0][0], 2], [0, GS]]))
                    a_ch = chpool.tile([128, 1], f32, tag="ach", name="a_ch")
                    nc.vector.tensor_mul(a_ch[:], rs_ch[:], gam_sb[:, ct:ct + 1])
                    b_ch = chpool.tile([128, 1], f32, tag="bch", name="b_ch")
                    nc.vector.tensor_mul(b_ch[:], mu_ch[:], a_ch[:])
                    nc.vector.tensor_sub(b_ch[:], bet_sb[:, ct:ct + 1], b_ch[:])
                    eng = nc.vector if ct % 2 == 0 else nc.gpsimd
                    eng.tensor_scalar(
                        out=xnt[:, ct // 2, ct % 2, :], in0=xt[:, ct, :],
                        scalar1=a_ch[:], scalar2=b_ch[:],
                        op0=OP.mult, op1=OP.add)
                return xnt

            def stage_qkv(img, xnt):
                """q,k (bf16, channel-major) + v (fp8, token-major, planes)."""
                qk = qkpool.tile([128, 2 * NH, N], bf16, tag="qk", name=f"qk{img}")
                # mt order pairs q_h with k_h so attn can start early
                for mt in (0, 4, 1, 5, 2, 6, 3, 7):
                    ps = mmps.tile([128, 2, 512], f32, tag="mm", name=f"qkps{mt}")
                    for kp in range(2):
                        for ch in range(2):
                            nc.tensor.matmul(
                                ps[:, ch, :],
                                wqk_sb[:, kp, :, mt * 128:(mt + 1) * 128],
                                xnt[:, kp, :, ch * 512:(ch + 1) * 512],
                                start=(kp == 0), stop=(kp == 1), perf_mode=DR)
                    if mt < 4:  # q: Scalar evac (unscale + bias + bf16 cast)
                        nc.scalar.activation(
                            qk[:, mt, :], ps[:, :, :], AF.Identity,
                            bias=qkb_sb[:, mt:mt + 1], scale=1.0 / WS)
                    else:       # k: DVE evac
                        nc.vector.tensor_scalar(
                            out=qk[:, mt, :], in0=ps[:, :, :],
                            scalar1=1.0 / WS, scalar2=qkb_sb[:, mt:mt + 1],
                            op0=OP.mult, op1=OP.add)

                vt = vtpool.tile([128, NT // 2, 2, C], fp8, tag="vt",
                                 name=f"vt{img}")
                for j in range(NT // 2):
                    ps = mmps.tile([128, 2, 512], f32, tag="mm", name=f"vps{j}")
                    for s in range(2):
                        nt = 2 * j + s
                        for kp in range(2):
                            nc.tensor.matmul(
                                ps[:, s, :],
                                xnt[:, kp, :, nt * 128:(nt + 1) * 128],
                                wv_sb[:, kp, :, :],
                                start=(kp == 0), stop=(kp == 1), perf_mode=DR)
                    nc.vector.scalar_tensor_tensor(
                        out=vt[:, j, :, :], in0=ps[:, :, :], scalar=1.0 / WS,
                        in1=vb_bc2[:, :, :], op0=OP.mult, op1=OP.add)
                return qk, vt

            def head_S(img, h, qk):
                """S^T = K^T Q (bf16) -> exp -> pt fp8 with mt-pair planes."""
                pt = ptpool.tile([128, NT // 2, 2, N], fp8, tag="pt",
                                 name=f"pt{img}_{h}")
                for mt in range(NT):
                    ps = mmps.tile([128, 2, 512], f32, tag="mm",
                                   name=f"sps{mt}")
                    for ch in range(2):
                        nc.tensor.matmul(
                            ps[:, ch, :],
                            qk[:, NH + h, mt * 128:(mt + 1) * 128],
                            qk[:, h, ch * 512:(ch + 1) * 512],
                            start=True, stop=True)
                    nc.scalar.activation(
                        pt[:, mt // 2, mt % 2, :], ps[:, :, :], AF.Exp,
                        bias=eb_t[:], scale=SCALE)
                return pt

            def head_RPV(img, h, pt, vt, ot):
                """rowsum + PV (both fp8 DoubleRow), then normalize."""
                pv = pvps.tile([128, 2, 512], f32, tag="pv", name="pv")
                rs = rsps.tile([1, 2, 512], f32, tag="rs", name="rs")
                for mp in range(NT // 2):
                    for ch in range(2):
                        nc.tensor.matmul(
                            rs[:, ch, :],
                            ones8[:, :, 0:1],
                            pt[:, mp, :, ch * 512:(ch + 1) * 512],
                            start=(mp == 0), stop=(mp == NT // 2 - 1),
                            perf_mode=DR)
                    for ch in range(2):
                        nc.tensor.matmul(
                            pv[:, ch, :],
                            vt[:, mp, :, h * 128:(h + 1) * 128],
                            pt[:, mp, :, ch * 512:(ch + 1) * 512],
                            start=(mp == 0), stop=(mp == NT // 2 - 1),
                            perf_mode=DR)
                rinv = rpool.tile([1, N], f32, tag="rinv", name="rinv", bufs=2)
                nc.vector.reciprocal_approx_fast(rinv[:], rs[0:1, :, :])
                rb = rpool.tile([128, N], f32, tag="rb", name="rb")
                nc.gpsimd.partition_broadcast(rb[:], rinv[:], channels=128)
                # ot = pv * rb  (= 16 * attnout, good fp8 range)
                nc.vector.tensor_mul(ot[:, h // 2, h % 2, :], pv[:, :, :], rb[:])

            def stage_attn(img, qk, vt, after_head=None):
                ot = otpool.tile([128, 2, 2, N], fp8, tag="ot", name=f"ot{img}")
                pts = {}
                pts[0] = head_S(img, 0, qk)
                for h in range(1, NH):
                    pts[h] = head_S(img, h, qk)
                    head_RPV(img, h - 1, pts[h - 1], vt, ot)
                    if after_head is not None:
                        after_head(h - 1)
                head_RPV(img, NH - 1, pts[NH - 1], vt, ot)
                if after_head is not None:
                    after_head(NH - 1)
                return ot

            # x + proj_bias, precomputed early on GpSimd so nothing queues
            # behind the rowsum broadcasts there
            rxpbs = []

            def stage_rxpb(img):
                rx = tpool.tile([128, CT, N], f32, tag="rxpb",
                                name=f"rxpb{img}")
                for t in range(CT):
                    nc.gpsimd.tensor_scalar(
                        out=rx[:, t, :], in0=xts[img][:, t, :],
                        scalar1=pb_sb[:, t:t + 1], scalar2=None, op0=OP.add)
                rxpbs.append(rx)

            def emit_proj(img, ot, t):
                ps = mmps.tile([128, 2, 512], f32, tag="mm", name=f"pps{t}")
                for hp in range(2):
                    for ch in range(2):
                        nc.tensor.matmul(
                            ps[:, ch, :],
                            wp_sb[:, hp, :, t * 128:(t + 1) * 128],
                            ot[:, hp, :, ch * 512:(ch + 1) * 512],
                            start=(hp == 0), stop=(hp == 1), perf_mode=DR)
                outt = outpool.tile([128, N], f32, tag="outt",
                                    name=f"o{img}_{t}")
                nc.vector.scalar_tensor_tensor(
                    out=outt[:], in0=ps[:, :, :], scalar=1.0 / (WS * 16.0),
                    in1=rxpbs[img][:, t, :], op0=OP.mult, op1=OP.add)
                nc.sync.dma_start(
                    out_d[img, t * 128:(t + 1) * 128, :], outt[:])

            # ---- software pipeline over the two images ----
            xn0 = stage_gn(0)
            stage_rxpb(0)
            qk0, vt0 = stage_qkv(0, xn0)
            xn1 = stage_gn(1)
            stage_rxpb(1)
            ot0 = stage_attn(0, qk0, vt0)
            qk1, vt1 = stage_qkv(1, xn1)
            # lag-2 zipper: proj0 chunk t is emitted two heads after ot0
            # became available, so S(h+1) never waits on proj psum release
            ot1 = stage_attn(1, qk1, vt1,
                             after_head=lambda h: (
                                 emit_proj(0, ot0, h - 2) if h >= 2 else None))
            for t in range(2, CT):
                emit_proj(0, ot0, t)
            for t in range(CT):
                emit_proj(1, ot1, t)

    nc.compile()
    return nc


_NC_CACHE = None


def _get_nc():
    global _NC_CACHE
    if _NC_CACHE is None:
        _NC_CACHE = build_program()
    return _NC_CACHE


def _host_prep(x, norm_gamma, norm_beta, qkv_w, qkv_b, proj_w, proj_b):
    import ml_dtypes
    f8 = ml_dtypes.float8_e4m3

    def pack_w(wT):  # [c=512, o] -> [128, 2, 2, o] fp8, prescaled
        o = wT.shape[1]
        return np.ascontiguousarray(
            (wT.reshape(2, 2, 128, o) * WS).transpose(2, 0, 1, 3)
        ).astype(f8)

    qkv_w = np.asarray(qkv_w, dtype=np.float32)
    proj_w = np.asarray(proj_w, dtype=np.float32)
    qkv_b = np.asarray(qkv_b, dtype=np.float32)
    common = {
        "wqk": pack_w(qkv_w[:2 * C].T),
        "wv": pack_w(qkv_w[2 * C:].T),
        "wp": pack_w(proj_w.T),
        "qkb": np.ascontiguousarray(qkv_b[:2 * C].reshape(-1, 128).T),
        "vb": np.ascontiguousarray(qkv_b[2 * C:]),
        "pb": np.ascontiguousarray(
            np.asarray(proj_b, dtype=np.float32).reshape(CT, 128).T),
        "gamma": np.ascontiguousarray(
            np.asarray(norm_gamma, dtype=np.float32).reshape(CT, 128).T),
        "beta": np.ascontiguousarray(
            np.asarray(norm_beta, dtype=np.float32).reshape(CT, 128).T),
    }
    xr = np.ascontiguousarray(np.asarray(x, dtype=np.float32).reshape(B, C, N))
    in_maps = []
    for c in range(N_CORES):
        m = dict(common)
        m["x"] = np.ascontiguousarray(xr[c * B_LOC:(c + 1) * B_LOC])
        in_maps.append(m)
    return in_maps


def run(inputs, trace=False):
    nc = _get_nc()
    in_maps = _host_prep(**inputs)
    res = None
    for attempt in range(3):
        try:
            res = run_bass_kernel_spmd(
                nc, in_maps, core_ids=list(range(N_CORES)), trace=trace)
            break
        except Exception:
            # rare transient NRT_EXEC_UNIT_UNRECOVERABLE on a cold device;
            # a re-run on the recovered device succeeds.
            if attempt == 2:
                raise
    parts = [res.results[c]["out"] for c in range(N_CORES)]
    out = np.concatenate(parts, axis=0).reshape(B, C, HH, WW)
    return out.astype(np.float32), res


def kernel(**inputs):
    out, _ = run(inputs, trace=False)
    return out


# revision 20
# speedup vs baseline: 1.2226x; 1.2226x over previous
"""Trainium2 Bass kernel for nn_AttentionBlock (GroupNorm + MHA + proj + residual).

Input  x: [16, 512, 32, 32] fp32.  8 NeuronCores, data-parallel over batch
(2 images per core).  Everything is hardcoded for these shapes.

fp8-e4m3 DoubleRow edition.  All matmuls except the score matmul run in
fp8 e4m3 with perf_mode=DoubleRow (K=256 per instruction, 0.5 cyc/col):
  - QKV and proj weights are host-prescaled by 16 (fp8-friendly range)
    and unscaled in the psum evacuation.
  - q,k are evacuated to bf16; the S^T = K^T Q matmul runs in bf16
    (full-rate, contraction d=128 can't double-row without a partition
    remap).
  - exp runs on the Scalar engine with bias -ln(4) so P/4 stays inside
    e4m3 range (max ~240); the /4 cancels in the softmax ratio because
    the SAME fp8 pt tensor feeds both the PV matmul and the rowsum.
  - rowsum via a skinny ones-lhsT DoubleRow matmul (out [1, n]); the
    reciprocal is broadcast to 128 partitions on GpSimd.
  - v is computed pre-transposed [m, c_v] as fp8 with mt-pair planes so
    PV contracts 256 tokens per instruction.
Engine split: exp + q-evac on Scalar; k-evac, v-evac, reciprocal,
ot-normalize, proj-unscale on DVE; GroupNorm apply, rowsum broadcast and
the residual add on GpSimd (no PSUM port there, SBUF-only work).
DMA: x + out on the Act HWDGE queue, weights + stores on the SP queue.

Numerics: scores are ~N(0,1) over 1024 keys => near-flat softmax, so fp8
error in q/k/v/xn washes out in the weighted average; tolerance is 2e-2
and this lands ~1e-3.
"""

import math
import numpy as np

import concourse.bass as bass
import concourse.bacc as bacc
import concourse.tile as tile
from concourse import mybir
from concourse.bass_utils import run_bass_kernel_spmd

N_CORES = 8
B, C, HH, WW = 16, 512, 32, 32
N = HH * WW            # 1024 tokens per image
NH, DH = 4, 128        # heads, head dim
G, GS = 8, 64          # groups, channels per group
B_LOC = B // N_CORES   # images per core
EPS = 1e-5
CT = C // 128          # 4 channel tiles
NT = N // 128          # 8 token tiles
SCALE = float(DH) ** -0.5
WS = 16.0              # host-side weight prescale
EB = -math.log(4.0)    # exp bias: pt = P/4, keeps e4m3 in range
OS = 1.0 / 16.0        # ones value: rowsum psum = sum(pt)/16

f32 = mybir.dt.float32
bf16 = mybir.dt.bfloat16
fp8 = mybir.dt.float8e4
AF = mybir.ActivationFunctionType
OP = mybir.AluOpType
DR = mybir.MatmulPerfMode.DoubleRow


def build_program():
    nc = bacc.Bacc("TRN2", target_bir_lowering=False, debug=False)

    x_d = nc.dram_tensor("x", [B_LOC, C, N], f32, kind="ExternalInput").ap()
    wqk_d = nc.dram_tensor("wqk", [128, 2, 2, 2 * C], fp8, kind="ExternalInput").ap()
    wv_d = nc.dram_tensor("wv", [128, 2, 2, C], fp8, kind="ExternalInput").ap()
    wp_d = nc.dram_tensor("wp", [128, 2, 2, C], fp8, kind="ExternalInput").ap()
    qkb_d = nc.dram_tensor("qkb", [128, 2 * C // 128], f32, kind="ExternalInput").ap()
    vb_d = nc.dram_tensor("vb", [C], f32, kind="ExternalInput").ap()
    pb_d = nc.dram_tensor("pb", [128, CT], f32, kind="ExternalInput").ap()
    gam_d = nc.dram_tensor("gamma", [128, CT], f32, kind="ExternalInput").ap()
    bet_d = nc.dram_tensor("beta", [128, CT], f32, kind="ExternalInput").ap()
    out_d = nc.dram_tensor("out", [B_LOC, C, N], f32, kind="ExternalOutput").ap()

    with tile.TileContext(nc) as tc:
        with (
            tc.tile_pool(name="wpool", bufs=1) as wpool,
            tc.tile_pool(name="xpool", bufs=2) as xpool,
            tc.tile_pool(name="xnpool", bufs=2) as xnpool,
            tc.tile_pool(name="qkpool", bufs=2) as qkpool,
            tc.tile_pool(name="vtpool", bufs=2) as vtpool,
            tc.tile_pool(name="ptpool", bufs=2) as ptpool,
            tc.tile_pool(name="otpool", bufs=2) as otpool,
            tc.tile_pool(name="rpool", bufs=2) as rpool,
            tc.tile_pool(name="tpool", bufs=2) as tpool,
            tc.tile_pool(name="outpool", bufs=2) as outpool,
            tc.tile_pool(name="spool", bufs=2) as spool,
            tc.tile_pool(name="chpool", bufs=4) as chpool,
            tc.tile_pool(name="mmps", bufs=2, space="PSUM") as mmps,
            tc.tile_pool(name="pvps", bufs=1, space="PSUM") as pvps,
            tc.tile_pool(name="rsps", bufs=1, space="PSUM") as rsps,
        ):
            # ---- x0 alone on the Act HWDGE queue (GN0 critical path);
            # wqk first then x1 + the rest on the SP queue ----
            xts = []
            for img in range(B_LOC):
                xt = xpool.tile([128, CT, N], f32, tag="x", name=f"xt{img}")
                xts.append(xt)
            wqk_sb = wpool.tile([128, 2, 2, 2 * C], fp8, tag="wqk")

            xr0 = x_d[0].rearrange("(t p) n -> p t n", p=128)
            for ct in range(CT):
                nc.scalar.dma_start(xts[0][:, ct, :], xr0[:, ct, :])
            nc.sync.dma_start(wqk_sb[:], wqk_d[:])
            xr1 = x_d[1].rearrange("(t p) n -> p t n", p=128)
            for ct in range(CT):
                nc.sync.dma_start(xts[1][:, ct, :], xr1[:, ct, :])
            wv_sb = wpool.tile([128, 2, 2, C], fp8, tag="wv")
            nc.sync.dma_start(wv_sb[:], wv_d[:])
            wp_sb = wpool.tile([128, 2, 2, C], fp8, tag="wp")
            nc.sync.dma_start(wp_sb[:], wp_d[:])

            qkb_sb = wpool.tile([128, 2 * C // 128], f32, tag="qkb")
            nc.sync.dma_start(qkb_sb[:], qkb_d[:])
            pb_sb = wpool.tile([128, CT], f32, tag="pb")
            nc.sync.dma_start(pb_sb[:], pb_d[:])
            gam_sb = wpool.tile([128, CT], f32, tag="gam")
            nc.sync.dma_start(gam_sb[:], gam_d[:])
            bet_sb = wpool.tile([128, CT], f32, tag="bet")
            nc.sync.dma_start(bet_sb[:], bet_d[:])
            # vb broadcast to all partitions, twice along free (nt-pair stt)
            vb_bc2 = wpool.tile([128, 2, C], f32, tag="vbbc")
            nc.sync.dma_start(
                vb_bc2[:],
                bass.AP(tensor=vb_d.tensor, offset=vb_d.offset,
                        ap=[[0, 128], [0, 2], [1, C]]))

            sel = wpool.tile([128, 2], f32, tag="sel")
            nc.vector.memset(sel[0:64, 0:1], 1.0 / GS)
            nc.vector.memset(sel[64:128, 0:1], 0.0)
            nc.vector.memset(sel[0:64, 1:2], 0.0)
            nc.vector.memset(sel[64:128, 1:2], 1.0 / GS)
            # [128, 2, 16]: k-subtile plane step must be 16B-aligned for
            # DoubleRow ldweights; only column 0 is used.
            ones8 = wpool.tile([128, 2, 16], fp8, tag="ones8")
            nc.vector.memset(ones8[:], OS)
            eps_t = wpool.tile([2, 1], f32, tag="eps")
            nc.vector.memset(eps_t[:], EPS)
            eb_t = wpool.tile([128, 1], f32, tag="eb")
            nc.vector.memset(eb_t[:], EB)

            def stage_gn(img):
                """GroupNorm stats on DVE, apply on GpSimd -> xn fp8."""
                xt = xts[img]
                stats2 = spool.tile([128, 2 * CT], f32, tag="stats2",
                                    name=f"stats2_{img}")
                for ct in range(CT):
                    st = spool.tile([128, 2, 6], f32, tag="bnst", name="st")
                    nc.vector.bn_stats(st[:, 0, :], xt[:, ct, 0:512])
                    nc.vector.bn_stats(st[:, 1, :], xt[:, ct, 512:1024])
                    mv = spool.tile([128, 2], f32, tag="mv", name="mv")
                    nc.vector.bn_aggr(mv[:], st[:])
                    nc.vector.tensor_copy(stats2[:, 2 * ct:2 * ct + 1], mv[:, 0:1])
                    nc.vector.tensor_mul(
                        stats2[:, 2 * ct + 1:2 * ct + 2], mv[:, 0:1], mv[:, 0:1])
                    nc.vector.tensor_add(
                        stats2[:, 2 * ct + 1:2 * ct + 2],
                        stats2[:, 2 * ct + 1:2 * ct + 2], mv[:, 1:2])
                psg_t = mmps.tile([128, 2, 512], f32, tag="mm", name="psg")
                psg = psg_t[0:2, 0, 0:2 * CT]
                nc.tensor.matmul(psg, sel[:], stats2[:], start=True, stop=True)
                gs = spool.tile([2, 2 * CT], f32, tag="gs", name="gs")
                nc.scalar.activation(gs[:], psg, AF.Copy)
                gs3 = gs[:].rearrange("p (t s) -> p t s", s=2)
                tmp = spool.tile([2, CT], f32, tag="gtmp", name="tmp")
                nc.vector.tensor_mul(tmp[:], gs3[:, :, 0], gs3[:, :, 0])
                var_g = spool.tile([2, CT], f32, tag="gvar", name="var_g")
                nc.vector.tensor_sub(var_g[:], gs3[:, :, 1], tmp[:])
                sd = spool.tile([2, CT], f32, tag="gsd", name="sd")
                nc.scalar.activation(sd[:], var_g[:], AF.Sqrt, bias=eps_t[:])
                rstd_g = spool.tile([2, CT], f32, tag="grstd", name="rstd_g")
                rscr = spool.tile([2, CT], f32, tag="grscr", name="rscr")
                nc.vector.reciprocal_approx_accurate(rstd_g[:], sd[:], rscr[:])

                xnt = xnpool.tile([128, 2, 2, N], fp8, tag="xn", name=f"xn{img}")
                for ct in range(CT):
                    mu_ch = chpool.tile([128, 1], f32, tag="much", name="mu_ch")
                    sg = gs[:, 2 * ct:2 * ct + 1]
                    nc.sync.dma_start(
                        mu_ch[:],
                        bass.AP(tensor=sg.tensor, offset=sg.offset,
                                ap=[[sg.ap[0][0], 2], [0, GS]]))
                    rs_ch = chpool.tile([128, 1], f32, tag="rsch", name="rs_ch")
                    sg = rstd_g[:, ct:ct + 1]
                    nc.sync.dma_start(
                        rs_ch[:],
                        bass.AP(tensor=sg.tensor, offset=sg.offset,
                                ap=[[sg.ap[0][0], 2], [0, GS]]))
                    a_ch = chpool.tile([128, 1], f32, tag="ach", name="a_ch")
                    nc.vector.tensor_mul(a_ch[:], rs_ch[:], gam_sb[:, ct:ct + 1])
                    b_ch = chpool.tile([128, 1], f32, tag="bch", name="b_ch")
                    nc.vector.tensor_mul(b_ch[:], mu_ch[:], a_ch[:])
                    nc.vector.tensor_sub(b_ch[:], bet_sb[:, ct:ct + 1], b_ch[:])
                    eng = nc.vector if ct % 2 == 0 else nc.gpsimd
                    eng.tensor_scalar(
                        out=xnt[:, ct // 2, ct % 2, :], in0=xt[:, ct, :],
                        scalar1=a_ch[:], scalar2=b_ch[:],
                        op0=OP.mult, op1=OP.add)
                return xnt

            def stage_qkv(img, xnt):
                """q,k (bf16, channel-major) + v (fp8, token-major, planes)."""
                qk = qkpool.tile([128, 2 * NH, N], bf16, tag="qk", name=f"qk{img}")
                # mt order pairs q_h with k_h so attn can start early
                for mt in (0, 4, 1, 5, 2, 6, 3, 7):
                    ps = mmps.tile([128, 2, 512], f32, tag="mm", name=f"qkps{mt}")
                    for kp in range(2):
                        for ch in range(2):
                            nc.tensor.matmul(
                                ps[:, ch, :],
                                wqk_sb[:, kp, :, mt * 128:(mt + 1) * 128],
                                xnt[:, kp, :, ch * 512:(ch + 1) * 512],
                                start=(kp == 0), stop=(kp == 1), perf_mode=DR)
                    if mt < 4:  # q: Scalar evac (unscale + bias + bf16 cast)
                        nc.scalar.activation(
                            qk[:, mt, :], ps[:, :, :], AF.Identity,
                            bias=qkb_sb[:, mt:mt + 1], scale=1.0 / WS)
                    else:       # k: DVE evac
                        nc.vector.tensor_scalar(
                            out=qk[:, mt, :], in0=ps[:, :, :],
                            scalar1=1.0 / WS, scalar2=qkb_sb[:, mt:mt + 1],
                            op0=OP.mult, op1=OP.add)

                vt = vtpool.tile([128, NT // 2, 2, C], fp8, tag="vt",
                                 name=f"vt{img}")
                for j in range(NT // 2):
                    ps = mmps.tile([128, 2, 512], f32, tag="mm", name=f"vps{j}")
                    for s in range(2):
                        nt = 2 * j + s
                        for kp in range(2):
                            nc.tensor.matmul(
                                ps[:, s, :],
                                xnt[:, kp, :, nt * 128:(nt + 1) * 128],
                                wv_sb[:, kp, :, :],
                                start=(kp == 0), stop=(kp == 1), perf_mode=DR)
                    nc.vector.scalar_tensor_tensor(
                        out=vt[:, j, :, :], in0=ps[:, :, :], scalar=1.0 / WS,
                        in1=vb_bc2[:, :, :], op0=OP.mult, op1=OP.add)
                return qk, vt

            def head_S(img, h, qk):
                """S^T = K^T Q (bf16) -> exp -> pt fp8 with mt-pair planes."""
                pt = ptpool.tile([128, NT // 2, 2, N], fp8, tag="pt",
                                 name=f"pt{img}_{h}")
                for mt in range(NT):
                    ps = mmps.tile([128, 2, 512], f32, tag="mm",
                                   name=f"sps{mt}")
                    for ch in range(2):
                        nc.tensor.matmul(
                            ps[:, ch, :],
                            qk[:, NH + h, mt * 128:(mt + 1) * 128],
                            qk[:, h, ch * 512:(ch + 1) * 512],
                            start=True, stop=True)
                    nc.scalar.activation(
                        pt[:, mt // 2, mt % 2, :], ps[:, :, :], AF.Exp,
                        bias=eb_t[:], scale=SCALE)
                return pt

            def head_RPV(img, h, pt, vt, ot):
                """rowsum + PV (both fp8 DoubleRow), then normalize."""
                pv = pvps.tile([128, 2, 512], f32, tag="pv", name="pv")
                rs = rsps.tile([1, 2, 512], f32, tag="rs", name="rs")
                for mp in range(NT // 2):
                    for ch in range(2):
                        nc.tensor.matmul(
                            rs[:, ch, :],
                            ones8[:, :, 0:1],
                            pt[:, mp, :, ch * 512:(ch + 1) * 512],
                            start=(mp == 0), stop=(mp == NT // 2 - 1),
                            perf_mode=DR)
                    for ch in range(2):
                        nc.tensor.matmul(
                            pv[:, ch, :],
                            vt[:, mp, :, h * 128:(h + 1) * 128],
                            pt[:, mp, :, ch * 512:(ch + 1) * 512],
                            start=(mp == 0), stop=(mp == NT // 2 - 1),
                            perf_mode=DR)
                rinv = rpool.tile([1, N], f32, tag="rinv", name="rinv", bufs=2)
                nc.vector.reciprocal_approx_fast(rinv[:], rs[0:1, :, :])
                rb = rpool.tile([128, N], f32, tag="rb", name="rb")
                nc.gpsimd.partition_broadcast(rb[:], rinv[:], channels=128)
                # ot = pv * rb  (= 16 * attnout, good fp8 range)
                nc.vector.tensor_mul(ot[:, h // 2, h % 2, :], pv[:, :, :], rb[:])

            def stage_attn(img, qk, vt, after_head=None):
                ot = otpool.tile([128, 2, 2, N], fp8, tag="ot", name=f"ot{img}")
                pts = {}
                pts[0] = head_S(img, 0, qk)
                for h in range(1, NH):
                    pts[h] = head_S(img, h, qk)
                    head_RPV(img, h - 1, pts[h - 1], vt, ot)
                    if after_head is not None:
                        after_head(h - 1)
                head_RPV(img, NH - 1, pts[NH - 1], vt, ot)
                if after_head is not None:
                    after_head(NH - 1)
                return ot

            # x + proj_bias, precomputed early on GpSimd so nothing queues
            # behind the rowsum broadcasts there
            rxpbs = []

            def stage_rxpb(img):
                rx = tpool.tile([128, CT, N], f32, tag="rxpb",
                                name=f"rxpb{img}")
                for t in range(CT):
                    eng = nc.gpsimd if t % 2 else nc.vector
                    eng.tensor_scalar(
                        out=rx[:, t, :], in0=xts[img][:, t, :],
                        scalar1=pb_sb[:, t:t + 1], scalar2=0.0,
                        op0=OP.add, op1=OP.add)
                rxpbs.append(rx)

            def emit_proj(img, ot, t):
                ps = mmps.tile([128, 2, 512], f32, tag="mm", name=f"pps{t}")
                for hp in range(2):
                    for ch in range(2):
                        nc.tensor.matmul(
                            ps[:, ch, :],
                            wp_sb[:, hp, :, t * 128:(t + 1) * 128],
                            ot[:, hp, :, ch * 512:(ch + 1) * 512],
                            start=(hp == 0), stop=(hp == 1), perf_mode=DR)
                outt = outpool.tile([128, N], f32, tag="outt",
                                    name=f"o{img}_{t}")
                nc.vector.scalar_tensor_tensor(
                    out=outt[:], in0=ps[:, :, :], scalar=1.0 / (WS * 16.0),
                    in1=rxpbs[img][:, t, :], op0=OP.mult, op1=OP.add)
                nc.sync.dma_start(
                    out_d[img, t * 128:(t + 1) * 128, :], outt[:])

            # ---- software pipeline over the two images ----
            xn0 = stage_gn(0)
            stage_rxpb(0)
            qk0, vt0 = stage_qkv(0, xn0)
            xn1 = stage_gn(1)
            stage_rxpb(1)
            ot0 = stage_attn(0, qk0, vt0)
            qk1, vt1 = stage_qkv(1, xn1)
            # lag-2 zipper: proj0 chunk t is emitted two heads after ot0
            # became available, so S(h+1) never waits on proj psum release
            ot1 = stage_attn(1, qk1, vt1,
                             after_head=lambda h: (
                                 emit_proj(0, ot0, h - 2) if h >= 2 else None))
            for t in range(2, CT):
                emit_proj(0, ot0, t)
            for t in range(CT):
                emit_proj(1, ot1, t)

    nc.compile()
    return nc


_NC_CACHE = None


def _get_nc():
    global _NC_CACHE
    if _NC_CACHE is None:
        _NC_CACHE = build_program()
    return _NC_CACHE


def _host_prep(x, norm_gamma, norm_beta, qkv_w, qkv_b, proj_w, proj_b):
    import ml_dtypes
    f8 = ml_dtypes.float8_e4m3

    def pack_w(wT):  # [c=512, o] -> [128, 2, 2, o] fp8, prescaled
        o = wT.shape[1]
        return np.ascontiguousarray(
            (wT.reshape(2, 2, 128, o) * WS).transpose(2, 0, 1, 3)
        ).astype(f8)

    qkv_w = np.asarray(qkv_w, dtype=np.float32)
    proj_w = np.asarray(proj_w, dtype=np.float32)
    qkv_b = np.asarray(qkv_b, dtype=np.float32)
    common = {
        "wqk": pack_w(qkv_w[:2 * C].T),
        "wv": pack_w(qkv_w[2 * C:].T),
        "wp": pack_w(proj_w.T),
        "qkb": np.ascontiguousarray(qkv_b[:2 * C].reshape(-1, 128).T),
        "vb": np.ascontiguousarray(qkv_b[2 * C:]),
        "pb": np.ascontiguousarray(
            np.asarray(proj_b, dtype=np.float32).reshape(CT, 128).T),
        "gamma": np.ascontiguousarray(
            np.asarray(norm_gamma, dtype=np.float32).reshape(CT, 128).T),
        "beta": np.ascontiguousarray(
            np.asarray(norm_beta, dtype=np.float32).reshape(CT, 128).T),
    }
    xr = np.ascontiguousarray(np.asarray(x, dtype=np.float32).reshape(B, C, N))
    in_maps = []
    for c in range(N_CORES):
        m = dict(common)
        m["x"] = np.ascontiguousarray(xr[c * B_LOC:(c + 1) * B_LOC])
        in_maps.append(m)
    return in_maps


def run(inputs, trace=False):
    nc = _get_nc()
    in_maps = _host_prep(**inputs)
    res = None
    for attempt in range(3):
        try:
            res = run_bass_kernel_spmd(
                nc, in_maps, core_ids=list(range(N_CORES)), trace=trace)
            break
        except Exception:
            # rare transient NRT_EXEC_UNIT_UNRECOVERABLE on a cold device;
            # a re-run on the recovered device succeeds.
            if attempt == 2:
                raise
    parts = [res.results[c]["out"] for c in range(N_CORES)]
    out = np.concatenate(parts, axis=0).reshape(B, C, HH, WW)
    return out.astype(np.float32), res


def kernel(**inputs):
    out, _ = run(inputs, trace=False)
    return out


# revision 25
# speedup vs baseline: 1.2375x; 1.0122x over previous
"""Trainium2 Bass kernel for nn_AttentionBlock (GroupNorm + MHA + proj + residual).

Input  x: [16, 512, 32, 32] fp32.  8 NeuronCores, data-parallel over batch
(2 images per core).  Everything is hardcoded for these shapes.

fp8-e4m3 DoubleRow edition.  All matmuls except the score matmul run in
fp8 e4m3 with perf_mode=DoubleRow (K=256 per instruction, 0.5 cyc/col):
  - QKV and proj weights are host-prescaled by 16 (fp8-friendly range)
    and unscaled in the psum evacuation.
  - q,k are evacuated to bf16; the S^T = K^T Q matmul runs in bf16
    (full-rate, contraction d=128 can't double-row without a partition
    remap).
  - exp runs on the Scalar engine with bias -ln(4) so P/4 stays inside
    e4m3 range (max ~240); the /4 cancels in the softmax ratio because
    the SAME fp8 pt tensor feeds both the PV matmul and the rowsum.
  - rowsum via a skinny ones-lhsT DoubleRow matmul (out [1, n]); the
    reciprocal is broadcast to 128 partitions on GpSimd.
  - v is computed pre-transposed [m, c_v] as fp8 with mt-pair planes so
    PV contracts 256 tokens per instruction.
Engine split: exp + q-evac on Scalar; k-evac, v-evac, reciprocal,
ot-normalize, proj-unscale on DVE; GroupNorm apply, rowsum broadcast and
the residual add on GpSimd (no PSUM port there, SBUF-only work).
DMA: x + out on the Act HWDGE queue, weights + stores on the SP queue.

Numerics: scores are ~N(0,1) over 1024 keys => near-flat softmax, so fp8
error in q/k/v/xn washes out in the weighted average; tolerance is 2e-2
and this lands ~1e-3.
"""

import math
import numpy as np

import concourse.bass as bass
import concourse.bacc as bacc
import concourse.tile as tile
from concourse import mybir
from concourse.bass_utils import run_bass_kernel_spmd

N_CORES = 8
B, C, HH, WW = 16, 512, 32, 32
N = HH * WW            # 1024 tokens per image
NH, DH = 4, 128        # heads, head dim
G, GS = 8, 64          # groups, channels per group
B_LOC = B // N_CORES   # images per core
EPS = 1e-5
CT = C // 128          # 4 channel tiles
NT = N // 128          # 8 token tiles
SCALE = float(DH) ** -0.5
WS = 16.0              # host-side weight prescale
EB = -math.log(4.0)    # exp bias: pt = P/4, keeps e4m3 in range
OS = 1.0 / 16.0        # ones value: rowsum psum = sum(pt)/16

f32 = mybir.dt.float32
bf16 = mybir.dt.bfloat16
fp8 = mybir.dt.float8e4
AF = mybir.ActivationFunctionType
OP = mybir.AluOpType
DR = mybir.MatmulPerfMode.DoubleRow


def build_program():
    nc = bacc.Bacc("TRN2", target_bir_lowering=False, debug=False)

    x_d = nc.dram_tensor("x", [B_LOC, C, N], f32, kind="ExternalInput").ap()
    wqk_d = nc.dram_tensor("wqk", [128, 2, 2, 2 * C], fp8, kind="ExternalInput").ap()
    wv_d = nc.dram_tensor("wv", [128, 2, 2, C], fp8, kind="ExternalInput").ap()
    wp_d = nc.dram_tensor("wp", [128, 2, 2, C], fp8, kind="ExternalInput").ap()
    qkb_d = nc.dram_tensor("qkb", [128, 2 * C // 128], f32, kind="ExternalInput").ap()
    vb_d = nc.dram_tensor("vb", [C], f32, kind="ExternalInput").ap()
    pb_d = nc.dram_tensor("pb", [128, CT], f32, kind="ExternalInput").ap()
    gam_d = nc.dram_tensor("gamma", [128, CT], f32, kind="ExternalInput").ap()
    bet_d = nc.dram_tensor("beta", [128, CT], f32, kind="ExternalInput").ap()
    out_d = nc.dram_tensor("out", [B_LOC, C, N], f32, kind="ExternalOutput").ap()

    with tile.TileContext(nc) as tc:
        with (
            tc.tile_pool(name="wpool", bufs=1) as wpool,
            tc.tile_pool(name="xpool", bufs=2) as xpool,
            tc.tile_pool(name="xnpool", bufs=2) as xnpool,
            tc.tile_pool(name="qkpool", bufs=2) as qkpool,
            tc.tile_pool(name="vtpool", bufs=2) as vtpool,
            tc.tile_pool(name="ptpool", bufs=2) as ptpool,
            tc.tile_pool(name="otpool", bufs=2) as otpool,
            tc.tile_pool(name="rpool", bufs=2) as rpool,
            tc.tile_pool(name="tpool", bufs=2) as tpool,
            tc.tile_pool(name="outpool", bufs=2) as outpool,
            tc.tile_pool(name="spool", bufs=2) as spool,
            tc.tile_pool(name="chpool", bufs=4) as chpool,
            tc.tile_pool(name="mmps", bufs=2, space="PSUM") as mmps,
            tc.tile_pool(name="pvps", bufs=1, space="PSUM") as pvps,
            tc.tile_pool(name="rsps", bufs=1, space="PSUM") as rsps,
        ):
            # ---- DMA rings (~150 GB/s effective each, FIFO per ring):
            # SP:  tiny scale/bias tensors first, x0 back half, weights,
            #      x1 last (late consumers); out stores later.
            # ACT: x0 front half; GN stat broadcasts land here too.
            xts = []
            for img in range(B_LOC):
                xt = xpool.tile([128, CT, N], f32, tag="x", name=f"xt{img}")
                xts.append(xt)

            qkb_sb = wpool.tile([128, 2 * C // 128], f32, tag="qkb")
            nc.sync.dma_start(qkb_sb[:], qkb_d[:])
            pb_sb = wpool.tile([128, CT], f32, tag="pb")
            nc.sync.dma_start(pb_sb[:], pb_d[:])
            gam_sb = wpool.tile([128, CT], f32, tag="gam")
            nc.sync.dma_start(gam_sb[:], gam_d[:])
            bet_sb = wpool.tile([128, CT], f32, tag="bet")
            nc.sync.dma_start(bet_sb[:], bet_d[:])
            # vb broadcast to all partitions, twice along free (nt-pair stt)
            vb_bc2 = wpool.tile([128, 2, C], f32, tag="vbbc")
            nc.sync.dma_start(
                vb_bc2[:],
                bass.AP(tensor=vb_d.tensor, offset=vb_d.offset,
                        ap=[[0, 128], [0, 2], [1, C]]))

            xr0 = x_d[0].rearrange("(t p) n -> p t n", p=128)
            nc.scalar.dma_start(xts[0][:, 0, :], xr0[:, 0, :])
            nc.scalar.dma_start(xts[0][:, 1, :], xr0[:, 1, :])
            nc.sync.dma_start(xts[0][:, 2, :], xr0[:, 2, :])
            nc.sync.dma_start(xts[0][:, 3, :], xr0[:, 3, :])

            wqk_sb = wpool.tile([128, 2, 2, 2 * C], fp8, tag="wqk")
            nc.sync.dma_start(wqk_sb[:], wqk_d[:])
            wv_sb = wpool.tile([128, 2, 2, C], fp8, tag="wv")
            nc.sync.dma_start(wv_sb[:], wv_d[:])
            wp_sb = wpool.tile([128, 2, 2, C], fp8, tag="wp")
            nc.sync.dma_start(wp_sb[:], wp_d[:])

            xr1 = x_d[1].rearrange("(t p) n -> p t n", p=128)
            for ct in range(CT):
                nc.sync.dma_start(xts[1][:, ct, :], xr1[:, ct, :])

            sel = wpool.tile([128, 2], f32, tag="sel")
            nc.vector.memset(sel[0:64, 0:1], 1.0 / GS)
            nc.vector.memset(sel[64:128, 0:1], 0.0)
            nc.vector.memset(sel[0:64, 1:2], 0.0)
            nc.vector.memset(sel[64:128, 1:2], 1.0 / GS)
            # [128, 2, 16]: k-subtile plane step must be 16B-aligned for
            # DoubleRow ldweights; only column 0 is used.
            ones8 = wpool.tile([128, 2, 16], fp8, tag="ones8")
            nc.vector.memset(ones8[:], OS)
            eps_t = wpool.tile([2, 1], f32, tag="eps")
            nc.vector.memset(eps_t[:], EPS)
            eb_t = wpool.tile([128, 1], f32, tag="eb")
            nc.vector.memset(eb_t[:], EB)

            def stage_gn(img):
                """GroupNorm stats on DVE, apply on GpSimd -> xn fp8."""
                xt = xts[img]
                stats2 = spool.tile([128, 2 * CT], f32, tag="stats2",
                                    name=f"stats2_{img}")
                for ct in range(CT):
                    st = spool.tile([128, 2, 6], f32, tag="bnst", name="st")
                    nc.vector.bn_stats(st[:, 0, :], xt[:, ct, 0:512])
                    nc.vector.bn_stats(st[:, 1, :], xt[:, ct, 512:1024])
                    mv = spool.tile([128, 2], f32, tag="mv", name="mv")
                    nc.vector.bn_aggr(mv[:], st[:])
                    nc.vector.tensor_copy(stats2[:, 2 * ct:2 * ct + 1], mv[:, 0:1])
                    nc.vector.tensor_mul(
                        stats2[:, 2 * ct + 1:2 * ct + 2], mv[:, 0:1], mv[:, 0:1])
                    nc.vector.tensor_add(
                        stats2[:, 2 * ct + 1:2 * ct + 2],
                        stats2[:, 2 * ct + 1:2 * ct + 2], mv[:, 1:2])
                psg_t = mmps.tile([128, 2, 512], f32, tag="mm", name="psg")
                psg = psg_t[0:2, 0, 0:2 * CT]
                nc.tensor.matmul(psg, sel[:], stats2[:], start=True, stop=True)
                gs = spool.tile([2, 2 * CT], f32, tag="gs", name="gs")
                nc.scalar.activation(gs[:], psg, AF.Copy)
                gs3 = gs[:].rearrange("p (t s) -> p t s", s=2)
                tmp = spool.tile([2, CT], f32, tag="gtmp", name="tmp")
                nc.vector.tensor_mul(tmp[:], gs3[:, :, 0], gs3[:, :, 0])
                var_g = spool.tile([2, CT], f32, tag="gvar", name="var_g")
                nc.vector.tensor_sub(var_g[:], gs3[:, :, 1], tmp[:])
                sd = spool.tile([2, CT], f32, tag="gsd", name="sd")
                nc.scalar.activation(sd[:], var_g[:], AF.Sqrt, bias=eps_t[:])
                rstd_g = spool.tile([2, CT], f32, tag="grstd", name="rstd_g")
                rscr = spool.tile([2, CT], f32, tag="grscr", name="rscr")
                nc.vector.reciprocal_approx_accurate(rstd_g[:], sd[:], rscr[:])

                xnt = xnpool.tile([128, 2, 2, N], fp8, tag="xn", name=f"xn{img}")
                for ct in range(CT):
                    mu_ch = chpool.tile([128, 1], f32, tag="much", name="mu_ch")
                    sg = gs[:, 2 * ct:2 * ct + 1]
                    nc.scalar.dma_start(
                        mu_ch[:],
                        bass.AP(tensor=sg.tensor, offset=sg.offset,
                                ap=[[sg.ap[0][0], 2], [0, GS]]))
                    rs_ch = chpool.tile([128, 1], f32, tag="rsch", name="rs_ch")
                    sg = rstd_g[:, ct:ct + 1]
                    nc.scalar.dma_start(
                        rs_ch[:],
                        bass.AP(tensor=sg.tensor, offset=sg.offset,
                                ap=[[sg.ap[0][0], 2], [0, GS]]))
                    a_ch = chpool.tile([128, 1], f32, tag="ach", name="a_ch")
                    nc.vector.tensor_mul(a_ch[:], rs_ch[:], gam_sb[:, ct:ct + 1])
                    b_ch = chpool.tile([128, 1], f32, tag="bch", name="b_ch")
                    nc.vector.tensor_mul(b_ch[:], mu_ch[:], a_ch[:])
                    nc.vector.tensor_sub(b_ch[:], bet_sb[:, ct:ct + 1], b_ch[:])
                    eng = nc.vector if ct % 2 == 0 else nc.gpsimd
                    eng.tensor_scalar(
                        out=xnt[:, ct // 2, ct % 2, :], in0=xt[:, ct, :],
                        scalar1=a_ch[:], scalar2=b_ch[:],
                        op0=OP.mult, op1=OP.add)
                return xnt

            def stage_qkv(img, xnt):
                """q,k (bf16, channel-major) + v (fp8, token-major, planes)."""
                qk = qkpool.tile([128, 2 * NH, N], bf16, tag="qk", name=f"qk{img}")
                # mt order pairs q_h with k_h so attn can start early
                for mt in (0, 4, 1, 5, 2, 6, 3, 7):
                    ps = mmps.tile([128, 2, 512], f32, tag="mm", name=f"qkps{mt}")
                    for kp in range(2):
                        for ch in range(2):
                            nc.tensor.matmul(
                                ps[:, ch, :],
                                wqk_sb[:, kp, :, mt * 128:(mt + 1) * 128],
                                xnt[:, kp, :, ch * 512:(ch + 1) * 512],
                                start=(kp == 0), stop=(kp == 1), perf_mode=DR)
                    if mt < 4:  # q: Scalar evac (unscale + bias + bf16 cast)
                        nc.scalar.activation(
                            qk[:, mt, :], ps[:, :, :], AF.Identity,
                            bias=qkb_sb[:, mt:mt + 1], scale=1.0 / WS)
                    else:       # k: DVE evac
                        nc.vector.tensor_scalar(
                            out=qk[:, mt, :], in0=ps[:, :, :],
                            scalar1=1.0 / WS, scalar2=qkb_sb[:, mt:mt + 1],
                            op0=OP.mult, op1=OP.add)

                vt = vtpool.tile([128, NT // 2, 2, C], fp8, tag="vt",
                                 name=f"vt{img}")
                for j in range(NT // 2):
                    ps = mmps.tile([128, 2, 512], f32, tag="mm", name=f"vps{j}")
                    for s in range(2):
                        nt = 2 * j + s
                        for kp in range(2):
                            nc.tensor.matmul(
                                ps[:, s, :],
                                xnt[:, kp, :, nt * 128:(nt + 1) * 128],
                                wv_sb[:, kp, :, :],
                                start=(kp == 0), stop=(kp == 1), perf_mode=DR)
                    nc.vector.scalar_tensor_tensor(
                        out=vt[:, j, :, :], in0=ps[:, :, :], scalar=1.0 / WS,
                        in1=vb_bc2[:, :, :], op0=OP.mult, op1=OP.add)
                return qk, vt

            def head_S(img, h, qk):
                """S^T = K^T Q (bf16) -> exp -> pt fp8 with mt-pair planes."""
                pt = ptpool.tile([128, NT // 2, 2, N], fp8, tag="pt",
                                 name=f"pt{img}_{h}")
                for mt in range(NT):
                    ps = mmps.tile([128, 2, 512], f32, tag="mm",
                                   name=f"sps{mt}")
                    for ch in range(2):
                        nc.tensor.matmul(
                            ps[:, ch, :],
                            qk[:, NH + h, mt * 128:(mt + 1) * 128],
                            qk[:, h, ch * 512:(ch + 1) * 512],
                            start=True, stop=True)
                    nc.scalar.activation(
                        pt[:, mt // 2, mt % 2, :], ps[:, :, :], AF.Exp,
                        bias=eb_t[:], scale=SCALE)
                return pt

            def head_RPV(img, h, pt, vt, ot):
                """rowsum + PV (both fp8 DoubleRow), then normalize."""
                pv = pvps.tile([128, 2, 512], f32, tag="pv", name="pv")
                rs = rsps.tile([1, 2, 512], f32, tag="rs", name="rs")
                for mp in range(NT // 2):
                    for ch in range(2):
                        nc.tensor.matmul(
                            rs[:, ch, :],
                            ones8[:, :, 0:1],
                            pt[:, mp, :, ch * 512:(ch + 1) * 512],
                            start=(mp == 0), stop=(mp == NT // 2 - 1),
                            perf_mode=DR)
                    for ch in range(2):
                        nc.tensor.matmul(
                            pv[:, ch, :],
                            vt[:, mp, :, h * 128:(h + 1) * 128],
                            pt[:, mp, :, ch * 512:(ch + 1) * 512],
                            start=(mp == 0), stop=(mp == NT // 2 - 1),
                            perf_mode=DR)
                rinv = rpool.tile([1, N], f32, tag="rinv", name="rinv", bufs=2)
                nc.vector.reciprocal_approx_fast(rinv[:], rs[0:1, :, :])
                rb = rpool.tile([128, N], f32, tag="rb", name="rb")
                nc.gpsimd.partition_broadcast(rb[:], rinv[:], channels=128)
                # ot = pv * rb  (= 16 * attnout, good fp8 range)
                nc.vector.tensor_mul(ot[:, h // 2, h % 2, :], pv[:, :, :], rb[:])

            def stage_attn(img, qk, vt, after_head=None):
                ot = otpool.tile([128, 2, 2, N], fp8, tag="ot", name=f"ot{img}")
                pts = {}
                pts[0] = head_S(img, 0, qk)
                for h in range(1, NH):
                    pts[h] = head_S(img, h, qk)
                    head_RPV(img, h - 1, pts[h - 1], vt, ot)
                    if after_head is not None:
                        after_head(h - 1)
                head_RPV(img, NH - 1, pts[NH - 1], vt, ot)
                if after_head is not None:
                    after_head(NH - 1)
                return ot

            # x += proj_bias, in place after GN consumed x; keeps the
            # residual-add out of the late proj chain
            def stage_rxpb(img):
                for t in range(CT):
                    eng = nc.gpsimd if t % 2 else nc.vector
                    eng.tensor_scalar(
                        out=xts[img][:, t, :], in0=xts[img][:, t, :],
                        scalar1=pb_sb[:, t:t + 1], scalar2=0.0,
                        op0=OP.add, op1=OP.add)

            def emit_proj(img, ot, t):
                ps = mmps.tile([128, 2, 512], f32, tag="mm", name=f"pps{t}")
                for hp in range(2):
                    for ch in range(2):
                        nc.tensor.matmul(
                            ps[:, ch, :],
                            wp_sb[:, hp, :, t * 128:(t + 1) * 128],
                            ot[:, hp, :, ch * 512:(ch + 1) * 512],
                            start=(hp == 0), stop=(hp == 1), perf_mode=DR)
                outt = outpool.tile([128, N], f32, tag="outt",
                                    name=f"o{img}_{t}")
                nc.vector.scalar_tensor_tensor(
                    out=outt[:], in0=ps[:, :, :], scalar=1.0 / (WS * 16.0),
                    in1=xts[img][:, t, :], op0=OP.mult, op1=OP.add)
                nc.sync.dma_start(
                    out_d[img, t * 128:(t + 1) * 128, :], outt[:])

            # ---- software pipeline over the two images ----
            xn0 = stage_gn(0)
            stage_rxpb(0)
            qk0, vt0 = stage_qkv(0, xn0)
            xn1 = stage_gn(1)
            stage_rxpb(1)
            ot0 = stage_attn(0, qk0, vt0)
            qk1, vt1 = stage_qkv(1, xn1)
            # lag-2 zipper: proj0 chunk t is emitted two heads after ot0
            # became available, so S(h+1) never waits on proj psum release
            ot1 = stage_attn(1, qk1, vt1,
                             after_head=lambda h: (
                                 emit_proj(0, ot0, h - 2) if h >= 2 else None))
            for t in range(2, CT):
                emit_proj(0, ot0, t)
            for t in range(CT):
                emit_proj(1, ot1, t)

    nc.compile()
    return nc


_NC_CACHE = None


def _get_nc():
    global _NC_CACHE
    if _NC_CACHE is None:
        _NC_CACHE = build_program()
    return _NC_CACHE


def _host_prep(x, norm_gamma, norm_beta, qkv_w, qkv_b, proj_w, proj_b):
    import ml_dtypes
    f8 = ml_dtypes.float8_e4m3

    def pack_w(wT):  # [c=512, o] -> [128, 2, 2, o] fp8, prescaled
        o = wT.shape[1]
        return np.ascontiguousarray(
            (wT.reshape(2, 2, 128, o) * WS).transpose(2, 0, 1, 3)
        ).astype(f8)

    qkv_w = np.asarray(qkv_w, dtype=np.float32)
    proj_w = np.asarray(proj_w, dtype=np.float32)
    qkv_b = np.asarray(qkv_b, dtype=np.float32)
    common = {
        "wqk": pack_w(qkv_w[:2 * C].T),
        "wv": pack_w(qkv_w[2 * C:].T),
        "wp": pack_w(proj_w.T),
        "qkb": np.ascontiguousarray(qkv_b[:2 * C].reshape(-1, 128).T),
        "vb": np.ascontiguousarray(qkv_b[2 * C:]),
        "pb": np.ascontiguousarray(
            np.asarray(proj_b, dtype=np.float32).reshape(CT, 128).T),
        "gamma": np.ascontiguousarray(
            np.asarray(norm_gamma, dtype=np.float32).reshape(CT, 128).T),
        "beta": np.ascontiguousarray(
            np.asarray(norm_beta, dtype=np.float32).reshape(CT, 128).T),
    }
    xr = np.ascontiguousarray(np.asarray(x, dtype=np.float32).reshape(B, C, N))
    in_maps = []
    for c in range(N_CORES):
        m = dict(common)
        m["x"] = np.ascontiguousarray(xr[c * B_LOC:(c + 1) * B_LOC])
        in_maps.append(m)
    return in_maps


def run(inputs, trace=False):
    nc = _get_nc()
    in_maps = _host_prep(**inputs)
    res = None
    for attempt in range(3):
        try:
            res = run_bass_kernel_spmd(
                nc, in_maps, core_ids=list(range(N_CORES)), trace=trace)
            break
        except Exception:
            # rare transient NRT_EXEC_UNIT_UNRECOVERABLE on a cold device;
            # a re-run on the recovered device succeeds.
            if attempt == 2:
                raise
    parts = [res.results[c]["out"] for c in range(N_CORES)]
    out = np.concatenate(parts, axis=0).reshape(B, C, HH, WW)
    return out.astype(np.float32), res


def kernel(**inputs):
    out, _ = run(inputs, trace=False)
    return out


# revision 26
# speedup vs baseline: 1.6361x; 1.3221x over previous
"""Trainium2 Bass kernel for nn_AttentionBlock (GroupNorm + MHA + proj + residual).

Input  x: [16, 512, 32, 32] fp32.  8 NeuronCores, data-parallel over batch
(2 images per core).  Everything is hardcoded for these shapes.

fp8-e4m3 DoubleRow edition.  All matmuls except the score matmul run in
fp8 e4m3 with perf_mode=DoubleRow (K=256 per instruction, 0.5 cyc/col):
  - QKV and proj weights are host-prescaled by 16 (fp8-friendly range)
    and unscaled in the psum evacuation.
  - q,k are evacuated to bf16; the S^T = K^T Q matmul runs in bf16
    (full-rate, contraction d=128 can't double-row without a partition
    remap).
  - exp runs on the Scalar engine with bias -ln(4) so P/4 stays inside
    e4m3 range (max ~240); the /4 cancels in the softmax ratio because
    the SAME fp8 pt tensor feeds both the PV matmul and the rowsum.
  - rowsum via a skinny ones-lhsT DoubleRow matmul (out [1, n]); the
    reciprocal is broadcast to 128 partitions on GpSimd.
  - v is computed pre-transposed [m, c_v] as fp8 with mt-pair planes so
    PV contracts 256 tokens per instruction.
Engine split: exp + q-evac on Scalar; k-evac, v-evac, reciprocal,
ot-normalize, proj-unscale on DVE; GroupNorm apply, rowsum broadcast and
the residual add on GpSimd (no PSUM port there, SBUF-only work).
DMA: x + out on the Act HWDGE queue, weights + stores on the SP queue.

Numerics: scores are ~N(0,1) over 1024 keys => near-flat softmax, so fp8
error in q/k/v/xn washes out in the weighted average; tolerance is 2e-2
and this lands ~1e-3.
"""

import math
import numpy as np

import concourse.bass as bass
import concourse.bacc as bacc
import concourse.tile as tile
from concourse import mybir
from concourse.bass_utils import run_bass_kernel_spmd

N_CORES = 8
B, C, HH, WW = 16, 512, 32, 32
N = HH * WW            # 1024 tokens per image
NH, DH = 4, 128        # heads, head dim
G, GS = 8, 64          # groups, channels per group
B_LOC = B // N_CORES   # images per core
EPS = 1e-5
CT = C // 128          # 4 channel tiles
NT = N // 128          # 8 token tiles
SCALE = float(DH) ** -0.5
WS = 16.0              # host-side weight prescale
EB = -math.log(4.0)    # exp bias: pt = P/4, keeps e4m3 in range
OS = 1.0 / 16.0        # ones value: rowsum psum = sum(pt)/16

f32 = mybir.dt.float32
bf16 = mybir.dt.bfloat16
fp8 = mybir.dt.float8e4
AF = mybir.ActivationFunctionType
OP = mybir.AluOpType
DR = mybir.MatmulPerfMode.DoubleRow


def build_program():
    nc = bacc.Bacc("TRN2", target_bir_lowering=False, debug=False)

    x_d = nc.dram_tensor("x", [B_LOC, C, N], f32, kind="ExternalInput").ap()
    wqk_d = nc.dram_tensor("wqk", [128, 2, 2, 2 * C], fp8, kind="ExternalInput").ap()
    wv_d = nc.dram_tensor("wv", [128, 2, 2, C], fp8, kind="ExternalInput").ap()
    wp_d = nc.dram_tensor("wp", [128, 2, 2, C], fp8, kind="ExternalInput").ap()
    qkb_d = nc.dram_tensor("qkb", [128, 2 * C // 128], f32, kind="ExternalInput").ap()
    vb_d = nc.dram_tensor("vb", [C], f32, kind="ExternalInput").ap()
    pb_d = nc.dram_tensor("pb", [128, CT], f32, kind="ExternalInput").ap()
    gam_d = nc.dram_tensor("gamma", [128, CT], f32, kind="ExternalInput").ap()
    bet_d = nc.dram_tensor("beta", [128, CT], f32, kind="ExternalInput").ap()
    out_d = nc.dram_tensor("out", [B_LOC, C, N], f32, kind="ExternalOutput").ap()

    with tile.TileContext(nc) as tc:
        with (
            tc.tile_pool(name="wpool", bufs=1) as wpool,
            tc.tile_pool(name="xpool", bufs=2) as xpool,
            tc.tile_pool(name="xnpool", bufs=2) as xnpool,
            tc.tile_pool(name="qkpool", bufs=2) as qkpool,
            tc.tile_pool(name="vtpool", bufs=2) as vtpool,
            tc.tile_pool(name="ptpool", bufs=2) as ptpool,
            tc.tile_pool(name="otpool", bufs=2) as otpool,
            tc.tile_pool(name="rpool", bufs=2) as rpool,
            tc.tile_pool(name="tpool", bufs=2) as tpool,
            tc.tile_pool(name="outpool", bufs=2) as outpool,
            tc.tile_pool(name="spool", bufs=2) as spool,
            tc.tile_pool(name="chpool", bufs=4) as chpool,
            tc.tile_pool(name="mmps", bufs=2, space="PSUM") as mmps,
            tc.tile_pool(name="pvps", bufs=1, space="PSUM") as pvps,
            tc.tile_pool(name="rsps", bufs=1, space="PSUM") as rsps,
        ):
            # ---- DMA rings (~150 GB/s effective each, FIFO per ring):
            # SP:  tiny scale/bias tensors first, x0 back half, weights,
            #      x1 last (late consumers); out stores later.
            # ACT: x0 front half; GN stat broadcasts land here too.
            xts = []
            for img in range(B_LOC):
                xt = xpool.tile([128, CT, N], f32, tag="x", name=f"xt{img}")
                xts.append(xt)

            qkb_sb = wpool.tile([128, 2 * C // 128], f32, tag="qkb")
            nc.sync.dma_start(qkb_sb[:], qkb_d[:])
            pb_sb = wpool.tile([128, CT], f32, tag="pb")
            nc.sync.dma_start(pb_sb[:], pb_d[:])
            gam_sb = wpool.tile([128, CT], f32, tag="gam")
            nc.sync.dma_start(gam_sb[:], gam_d[:])
            bet_sb = wpool.tile([128, CT], f32, tag="bet")
            nc.sync.dma_start(bet_sb[:], bet_d[:])
            # vb broadcast to all partitions, twice along free (nt-pair stt)
            vb_bc2 = wpool.tile([128, 2, C], f32, tag="vbbc")
            nc.sync.dma_start(
                vb_bc2[:],
                bass.AP(tensor=vb_d.tensor, offset=vb_d.offset,
                        ap=[[0, 128], [0, 2], [1, C]]))

            xr0 = x_d[0].rearrange("(t p) n -> p t n", p=128)
            nc.scalar.dma_start(xts[0][:, 0, :], xr0[:, 0, :])
            nc.scalar.dma_start(xts[0][:, 1, :], xr0[:, 1, :])
            nc.sync.dma_start(xts[0][:, 2, :], xr0[:, 2, :])
            nc.sync.dma_start(xts[0][:, 3, :], xr0[:, 3, :])

            wqk_sb = wpool.tile([128, 2, 2, 2 * C], fp8, tag="wqk")
            nc.sync.dma_start(wqk_sb[:], wqk_d[:])
            wv_sb = wpool.tile([128, 2, 2, C], fp8, tag="wv")
            nc.sync.dma_start(wv_sb[:], wv_d[:])
            wp_sb = wpool.tile([128, 2, 2, C], fp8, tag="wp")
            nc.sync.dma_start(wp_sb[:], wp_d[:])

            xr1 = x_d[1].rearrange("(t p) n -> p t n", p=128)
            for ct in range(CT):
                nc.sync.dma_start(xts[1][:, ct, :], xr1[:, ct, :])

            sel = wpool.tile([128, 2], f32, tag="sel")
            nc.vector.memset(sel[0:64, 0:1], 1.0 / GS)
            nc.vector.memset(sel[64:128, 0:1], 0.0)
            nc.vector.memset(sel[0:64, 1:2], 0.0)
            nc.vector.memset(sel[64:128, 1:2], 1.0 / GS)
            # [128, 2, 16]: k-subtile plane step must be 16B-aligned for
            # DoubleRow ldweights; only column 0 is used.
            ones8 = wpool.tile([128, 2, 16], fp8, tag="ones8")
            nc.vector.memset(ones8[:], OS)
            eps_t = wpool.tile([2, 1], f32, tag="eps")
            nc.vector.memset(eps_t[:], EPS)
            eb_t = wpool.tile([128, 1], f32, tag="eb")
            nc.vector.memset(eb_t[:], EB)

            def stage_gn(img):
                """GroupNorm stats on DVE, apply on GpSimd -> xn fp8."""
                xt = xts[img]
                stats2 = spool.tile([128, 2 * CT], f32, tag="stats2",
                                    name=f"stats2_{img}")
                for ct in range(CT):
                    st = spool.tile([128, 2, 6], f32, tag="bnst", name="st")
                    nc.vector.bn_stats(st[:, 0, :], xt[:, ct, 0:512])
                    nc.vector.bn_stats(st[:, 1, :], xt[:, ct, 512:1024])
                    mv = spool.tile([128, 2], f32, tag="mv", name="mv")
                    nc.vector.bn_aggr(mv[:], st[:])
                    nc.vector.tensor_copy(stats2[:, 2 * ct:2 * ct + 1], mv[:, 0:1])
                    nc.vector.tensor_mul(
                        stats2[:, 2 * ct + 1:2 * ct + 2], mv[:, 0:1], mv[:, 0:1])
                    nc.vector.tensor_add(
                        stats2[:, 2 * ct + 1:2 * ct + 2],
                        stats2[:, 2 * ct + 1:2 * ct + 2], mv[:, 1:2])
                psg_t = mmps.tile([128, 2, 512], f32, tag="mm", name="psg")
                psg = psg_t[0:2, 0, 0:2 * CT]
                nc.tensor.matmul(psg, sel[:], stats2[:], start=True, stop=True)
                gs = spool.tile([2, 2 * CT], f32, tag="gs", name="gs")
                nc.scalar.activation(gs[:], psg, AF.Copy)
                gs3 = gs[:].rearrange("p (t s) -> p t s", s=2)
                tmp = spool.tile([2, CT], f32, tag="gtmp", name="tmp")
                nc.vector.tensor_mul(tmp[:], gs3[:, :, 0], gs3[:, :, 0])
                var_g = spool.tile([2, CT], f32, tag="gvar", name="var_g")
                nc.vector.tensor_sub(var_g[:], gs3[:, :, 1], tmp[:])
                sd = spool.tile([2, CT], f32, tag="gsd", name="sd")
                nc.scalar.activation(sd[:], var_g[:], AF.Sqrt, bias=eps_t[:])
                rstd_g = spool.tile([2, CT], f32, tag="grstd", name="rstd_g")
                rscr = spool.tile([2, CT], f32, tag="grscr", name="rscr")
                nc.vector.reciprocal_approx_accurate(rstd_g[:], sd[:], rscr[:])

                xnt = xnpool.tile([128, 2, 2, N], fp8, tag="xn", name=f"xn{img}")
                for ct in range(CT):
                    mu_ch = chpool.tile([128, 1], f32, tag="much", name="mu_ch")
                    sg = gs[:, 2 * ct:2 * ct + 1]
                    nc.scalar.dma_start(
                        mu_ch[:],
                        bass.AP(tensor=sg.tensor, offset=sg.offset,
                                ap=[[sg.ap[0][0], 2], [0, GS]]))
                    rs_ch = chpool.tile([128, 1], f32, tag="rsch", name="rs_ch")
                    sg = rstd_g[:, ct:ct + 1]
                    nc.scalar.dma_start(
                        rs_ch[:],
                        bass.AP(tensor=sg.tensor, offset=sg.offset,
                                ap=[[sg.ap[0][0], 2], [0, GS]]))
                    a_ch = chpool.tile([128, 1], f32, tag="ach", name="a_ch")
                    nc.vector.tensor_mul(a_ch[:], rs_ch[:], gam_sb[:, ct:ct + 1])
                    b_ch = chpool.tile([128, 1], f32, tag="bch", name="b_ch")
                    nc.vector.tensor_mul(b_ch[:], mu_ch[:], a_ch[:])
                    nc.vector.tensor_sub(b_ch[:], bet_sb[:, ct:ct + 1], b_ch[:])
                    eng = nc.vector if ct % 2 == 0 else nc.gpsimd
                    eng.tensor_scalar(
                        out=xnt[:, ct // 2, ct % 2, :], in0=xt[:, ct, :],
                        scalar1=a_ch[:], scalar2=b_ch[:],
                        op0=OP.mult, op1=OP.add)
                return xnt

            def stage_qkv(img, xnt):
                """q,k (bf16, channel-major) + v (fp8, token-major, planes)."""
                qk = qkpool.tile([128, 2 * NH, N], bf16, tag="qk", name=f"qk{img}")
                # mt order pairs q_h with k_h so attn can start early
                for mt in (0, 4, 1, 5, 2, 6, 3, 7):
                    ps = mmps.tile([128, 2, 512], f32, tag="mm", name=f"qkps{mt}")
                    for kp in range(2):
                        for ch in range(2):
                            nc.tensor.matmul(
                                ps[:, ch, :],
                                wqk_sb[:, kp, :, mt * 128:(mt + 1) * 128],
                                xnt[:, kp, :, ch * 512:(ch + 1) * 512],
                                start=(kp == 0), stop=(kp == 1), perf_mode=DR)
                    if mt < 4:  # q: Scalar evac (unscale + bias + bf16 cast)
                        nc.scalar.activation(
                            qk[:, mt, :], ps[:, :, :], AF.Identity,
                            bias=qkb_sb[:, mt:mt + 1], scale=1.0 / WS)
                    else:       # k: DVE evac
                        nc.vector.tensor_scalar(
                            out=qk[:, mt, :], in0=ps[:, :, :],
                            scalar1=1.0 / WS, scalar2=qkb_sb[:, mt:mt + 1],
                            op0=OP.mult, op1=OP.add)

                vt = vtpool.tile([128, NT // 2, 2, C], fp8, tag="vt",
                                 name=f"vt{img}")
                for j in range(NT // 2):
                    ps = mmps.tile([128, 2, 512], f32, tag="mm", name=f"vps{j}")
                    for s in range(2):
                        nt = 2 * j + s
                        for kp in range(2):
                            nc.tensor.matmul(
                                ps[:, s, :],
                                xnt[:, kp, :, nt * 128:(nt + 1) * 128],
                                wv_sb[:, kp, :, :],
                                start=(kp == 0), stop=(kp == 1), perf_mode=DR)
                    nc.vector.scalar_tensor_tensor(
                        out=vt[:, j, :, :], in0=ps[:, :, :], scalar=1.0 / WS,
                        in1=vb_bc2[:, :, :], op0=OP.mult, op1=OP.add)
                return qk, vt

            def head_S(img, h, qk):
                """S^T = K^T Q (bf16) -> exp -> pt fp8 with mt-pair planes."""
                pt = ptpool.tile([128, NT // 2, 2, N], fp8, tag="pt",
                                 name=f"pt{img}_{h}")
                for mt in range(NT):
                    ps = mmps.tile([128, 2, 512], f32, tag="mm",
                                   name=f"sps{mt}")
                    for ch in range(2):
                        nc.tensor.matmul(
                            ps[:, ch, :],
                            qk[:, NH + h, mt * 128:(mt + 1) * 128],
                            qk[:, h, ch * 512:(ch + 1) * 512],
                            start=True, stop=True)
                    nc.scalar.activation(
                        pt[:, mt // 2, mt % 2, :], ps[:, :, :], AF.Exp,
                        bias=eb_t[:], scale=SCALE)
                return pt

            def head_RPV(img, h, pt, vt, ot):
                """rowsum + PV (both fp8 DoubleRow), then normalize."""
                pv = pvps.tile([128, 2, 512], f32, tag="pv", name="pv")
                rs = rsps.tile([1, 2, 512], f32, tag="rs", name="rs")
                for mp in range(NT // 2):
                    for ch in range(2):
                        nc.tensor.matmul(
                            rs[:, ch, :],
                            ones8[:, :, 0:1],
                            pt[:, mp, :, ch * 512:(ch + 1) * 512],
                            start=(mp == 0), stop=(mp == NT // 2 - 1),
                            perf_mode=DR)
                    for ch in range(2):
                        nc.tensor.matmul(
                            pv[:, ch, :],
                            vt[:, mp, :, h * 128:(h + 1) * 128],
                            pt[:, mp, :, ch * 512:(ch + 1) * 512],
                            start=(mp == 0), stop=(mp == NT // 2 - 1),
                            perf_mode=DR)
                rinv = rpool.tile([1, N], f32, tag="rinv", name="rinv", bufs=2)
                nc.vector.reciprocal_approx_fast(rinv[:], rs[0:1, :, :])
                rb = rpool.tile([128, N], f32, tag="rb", name="rb")
                nc.gpsimd.partition_broadcast(rb[:], rinv[:], channels=128)
                # ot = pv * rb  (= 16 * attnout, good fp8 range)
                nc.vector.tensor_mul(ot[:, h // 2, h % 2, :], pv[:, :, :], rb[:])

            def stage_attn(img, qk, vt, after_head=None):
                ot = otpool.tile([128, 2, 2, N], fp8, tag="ot", name=f"ot{img}")
                pts = {}
                pts[0] = head_S(img, 0, qk)
                for h in range(1, NH):
                    pts[h] = head_S(img, h, qk)
                    head_RPV(img, h - 1, pts[h - 1], vt, ot)
                    if after_head is not None:
                        after_head(h - 1)
                head_RPV(img, NH - 1, pts[NH - 1], vt, ot)
                if after_head is not None:
                    after_head(NH - 1)
                return ot

            # x += proj_bias, in place after GN consumed x; keeps the
            # residual-add out of the late proj chain
            def stage_rxpb(img):
                for t in range(CT):
                    # DVE only: GpSimd is ~12x slower for f32-out ts
                    nc.vector.tensor_scalar(
                        out=xts[img][:, t, :], in0=xts[img][:, t, :],
                        scalar1=pb_sb[:, t:t + 1], scalar2=0.0,
                        op0=OP.add, op1=OP.add)

            def emit_proj(img, ot, t):
                ps = mmps.tile([128, 2, 512], f32, tag="mm", name=f"pps{t}")
                for hp in range(2):
                    for ch in range(2):
                        nc.tensor.matmul(
                            ps[:, ch, :],
                            wp_sb[:, hp, :, t * 128:(t + 1) * 128],
                            ot[:, hp, :, ch * 512:(ch + 1) * 512],
                            start=(hp == 0), stop=(hp == 1), perf_mode=DR)
                outt = outpool.tile([128, N], f32, tag="outt",
                                    name=f"o{img}_{t}")
                nc.vector.scalar_tensor_tensor(
                    out=outt[:], in0=ps[:, :, :], scalar=1.0 / (WS * 16.0),
                    in1=xts[img][:, t, :], op0=OP.mult, op1=OP.add)
                nc.sync.dma_start(
                    out_d[img, t * 128:(t + 1) * 128, :], outt[:])

            # ---- software pipeline over the two images ----
            xn0 = stage_gn(0)
            stage_rxpb(0)
            qk0, vt0 = stage_qkv(0, xn0)
            xn1 = stage_gn(1)
            stage_rxpb(1)
            ot0 = stage_attn(0, qk0, vt0)
            qk1, vt1 = stage_qkv(1, xn1)
            # lag-2 zipper: proj0 chunk t is emitted two heads after ot0
            # became available, so S(h+1) never waits on proj psum release
            ot1 = stage_attn(1, qk1, vt1,
                             after_head=lambda h: (
                                 emit_proj(0, ot0, h - 2) if h >= 2 else None))
            for t in range(2, CT):
                emit_proj(0, ot0, t)
            for t in range(CT):
                emit_proj(1, ot1, t)

    nc.compile()
    return nc


_NC_CACHE = None


def _get_nc():
    global _NC_CACHE
    if _NC_CACHE is None:
        _NC_CACHE = build_program()
    return _NC_CACHE


def _host_prep(x, norm_gamma, norm_beta, qkv_w, qkv_b, proj_w, proj_b):
    import ml_dtypes
    f8 = ml_dtypes.float8_e4m3

    def pack_w(wT):  # [c=512, o] -> [128, 2, 2, o] fp8, prescaled
        o = wT.shape[1]
        return np.ascontiguousarray(
            (wT.reshape(2, 2, 128, o) * WS).transpose(2, 0, 1, 3)
        ).astype(f8)

    qkv_w = np.asarray(qkv_w, dtype=np.float32)
    proj_w = np.asarray(proj_w, dtype=np.float32)
    qkv_b = np.asarray(qkv_b, dtype=np.float32)
    common = {
        "wqk": pack_w(qkv_w[:2 * C].T),
        "wv": pack_w(qkv_w[2 * C:].T),
        "wp": pack_w(proj_w.T),
        "qkb": np.ascontiguousarray(qkv_b[:2 * C].reshape(-1, 128).T),
        "vb": np.ascontiguousarray(qkv_b[2 * C:]),
        "pb": np.ascontiguousarray(
            np.asarray(proj_b, dtype=np.float32).reshape(CT, 128).T),
        "gamma": np.ascontiguousarray(
            np.asarray(norm_gamma, dtype=np.float32).reshape(CT, 128).T),
        "beta": np.ascontiguousarray(
            np.asarray(norm_beta, dtype=np.float32).reshape(CT, 128).T),
    }
    xr = np.ascontiguousarray(np.asarray(x, dtype=np.float32).reshape(B, C, N))
    in_maps = []
    for c in range(N_CORES):
        m = dict(common)
        m["x"] = np.ascontiguousarray(xr[c * B_LOC:(c + 1) * B_LOC])
        in_maps.append(m)
    return in_maps


def run(inputs, trace=False):
    nc = _get_nc()
    in_maps = _host_prep(**inputs)
    res = None
    for attempt in range(3):
        try:
            res = run_bass_kernel_spmd(
                nc, in_maps, core_ids=list(range(N_CORES)), trace=trace)
            break
        except Exception:
            # rare transient NRT_EXEC_UNIT_UNRECOVERABLE on a cold device;
            # a re-run on the recovered device succeeds.
            if attempt == 2:
                raise
    parts = [res.results[c]["out"] for c in range(N_CORES)]
    out = np.concatenate(parts, axis=0).reshape(B, C, HH, WW)
    return out.astype(np.float32), res


def kernel(**inputs):
    out, _ = run(inputs, trace=False)
    return out
